# revision 16
# baseline (speedup 1.0000x reference)
"""Multi-head attention TRN2 kernel (8 NeuronCores).

Sharding: data parallel on batch (B=2, 4 cores each), tensor parallel on
heads (4 of 16 heads per core; wq/wk/wv column-parallel, wo row-parallel).
Each core computes a partial [D, S] transposed output for its batch; the
host sums the 4 partials per batch, transposes, and adds bo.

Per-core device pipeline (all matmuls fp32r, full PE rate):
  1. Q^T, K^T = (wq_c^T @ x^T) in [depth-major, seq] layout, 1/8 scale and
     bias folded in. V = x @ wv_c in [seq, depth] layout (rhs doubled to
     N=512 to keep PE duty high), augmented with a ones column per head
     (softmax denominator comes out of the attn@V matmul).
  2. Attention per head pair / 512-query tile / 128-key-block PAIR
     (causal pairs skipped), software-pipelined: logits^T into a
     [128,1024] two-bank PSUM tile, optional additive mask (DVE), exp
     (ACT, one op per pair) -> attn^T; attn@V accumulates out^T [65,512]
     in PSUM (row 64 = denominator).
  3. Batched normalize: denominator rows collected into [128,64] via
     reshaping DMAs, one DVE reciprocal, per-head gather -> GPSIMD
     partition_broadcast -> two in-place [128,2048] multiplies.
  4. Output projection: partial^T [D, S] = wo_c^T-chunks @ O^T.
"""

import numpy as np

import concourse.bass as bass
import concourse.mybir as mybir
import concourse.tile as tile
from concourse import bacc
from concourse.bass_utils import run_bass_kernel_spmd

B = 2
S = 2048
D_MODEL = 1024
NUM_HEADS = 16
DEPTH = 64
NEG = -1e9
N_CORES = 8
CORES_PER_BATCH = 4
HEADS_PER_CORE = 4           # 4 heads x depth 64 = 256 d_out columns per core
DC = HEADS_PER_CORE * DEPTH  # 256
QT = 512                     # query tile (4 tiles)
KB = 128                     # key block (16 blocks, processed in pairs)
NQT = S // QT
NKB = S // KB
NPAIR = NKB // 2
KIN = D_MODEL // 128         # 8 contraction chunks of 128

F32 = mybir.dt.float32
F32R = mybir.dt.float32r

_cache = {}


def _build(pair_plan, n_masks):
    """pair_plan[(t, pj)] = (valid0, valid1, mask_idx|None)."""
    nc = bacc.Bacc("TRN2", target_bir_lowering=False, debug=False,
                   num_devices=N_CORES)

    xqT = nc.dram_tensor("xqT", [D_MODEL, S], F32, kind="ExternalInput").ap()
    xkT = nc.dram_tensor("xkT", [D_MODEL, S], F32, kind="ExternalInput").ap()
    xvT = nc.dram_tensor("xvT", [D_MODEL, S], F32, kind="ExternalInput").ap()
    wq = nc.dram_tensor("wq", [D_MODEL, DC], F32, kind="ExternalInput").ap()
    wk = nc.dram_tensor("wk", [D_MODEL, DC], F32, kind="ExternalInput").ap()
    wv = nc.dram_tensor("wv", [D_MODEL, DC], F32, kind="ExternalInput").ap()
    wo = nc.dram_tensor("wo", [DC, D_MODEL], F32, kind="ExternalInput").ap()
    bq = nc.dram_tensor("bq", [128, 2], F32, kind="ExternalInput").ap()
    bk = nc.dram_tensor("bk", [128, 2], F32, kind="ExternalInput").ap()
    bv = nc.dram_tensor("bv", [128, DC], F32, kind="ExternalInput").ap()
    masks = nc.dram_tensor("masks", [max(n_masks, 1), KB, 2 * QT], F32,
                           kind="ExternalInput").ap()
    outT = nc.dram_tensor("outT", [D_MODEL, S], F32, kind="ExternalOutput").ap()

    with tile.TileContext(nc) as tc:
        import contextlib
        ctx = contextlib.ExitStack()
        with ctx:
            wpool = ctx.enter_context(tc.tile_pool(name="weights", bufs=1))
            qkv = ctx.enter_context(tc.tile_pool(name="qkv", bufs=1))
            xin = ctx.enter_context(tc.tile_pool(name="xin", bufs=7))
            expp = ctx.enter_context(tc.tile_pool(name="expp", bufs=4))
            ostp = ctx.enter_context(tc.tile_pool(name="ostp", bufs=3))
            nrmp = ctx.enter_context(tc.tile_pool(name="nrmp", bufs=1))
            rowp = ctx.enter_context(tc.tile_pool(name="rowp", bufs=1))
            psbig = ctx.enter_context(
                tc.tile_pool(name="psbig", bufs=3, space="PSUM"))
            pso = ctx.enter_context(
                tc.tile_pool(name="pso", bufs=2, space="PSUM"))

            # ---- resident weights / constants ------------------------------
            wq_sb = wpool.tile([128, KIN, DC], F32R, tag="wq")
            wk_sb = wpool.tile([128, KIN, DC], F32R, tag="wk")
            wv_sb = wpool.tile([128, KIN, DC], F32R, tag="wv")
            for c in range(KIN):
                nc.sync.dma_start(wq_sb[:, c, :], wq[c * 128:(c + 1) * 128, :].bitcast(F32R))
                nc.sync.dma_start(wk_sb[:, c, :], wk[c * 128:(c + 1) * 128, :].bitcast(F32R))
                nc.sync.dma_start(wv_sb[:, c, :], wv[c * 128:(c + 1) * 128, :].bitcast(F32R))
            wo_sb = wpool.tile([128, 2, D_MODEL], F32R, tag="wo")
            for c in range(2):
                nc.sync.dma_start(wo_sb[:, c, :], wo[c * 128:(c + 1) * 128, :].bitcast(F32R))

            bq_sb = wpool.tile([128, 2], F32, tag="bq")
            bk_sb = wpool.tile([128, 2], F32, tag="bk")
            bv_sb = wpool.tile([128, DC], F32, tag="bv")
            nc.sync.dma_start(bq_sb[:], bq[:])
            nc.sync.dma_start(bk_sb[:], bk[:])
            nc.sync.dma_start(bv_sb[:], bv[:])

            mask_sb = []
            for i in range(n_masks):
                mt = wpool.tile([KB, 2 * QT], F32, tag=f"mask{i}",
                                name=f"mask{i}")
                nc.sync.dma_start(mt[:], masks[i])
                mask_sb.append(mt)

            # persistent activations
            qt_sb = [qkv.tile([128, S], F32R, tag=f"qt{i}", name=f"qt{i}")
                     for i in range(2)]
            kt_sb = [qkv.tile([128, S], F32R, tag=f"kt{i}", name=f"kt{i}")
                     for i in range(2)]
            v_sb = qkv.tile([128, NKB, HEADS_PER_CORE, DEPTH + 1], F32R,
                            tag="v")
            ot_sb = [qkv.tile([128, S], F32R, tag=f"ot{i}", name=f"ot{i}")
                     for i in range(2)]

            # ones columns of V (denominator trick)
            ones_f32 = wpool.tile([128, 1], F32, tag="ones")
            nc.vector.memset(ones_f32[:], 1.0)
            nc.vector.tensor_copy(
                v_sb[:, :, :, DEPTH:DEPTH + 1],
                ones_f32[:, None, None, :].broadcast_to(
                    [128, NKB, HEADS_PER_CORE, 1]))

            # denominator staging: rs collects raw denoms, rr = 1/rs
            rs_sb = nrmp.tile([128, HEADS_PER_CORE * NQT * 4], F32, tag="rs")
            rr_sb = nrmp.tile([128, HEADS_PER_CORE * NQT * 4], F32, tag="rr")

            # ---- stage 1: projections --------------------------------------
            # Q^T / K^T: [d_out 128-tile, seq] = wq_chunk^T @ x^T_chunk
            # V interleaved to keep the PE stream dense.
            def emit_v_group(scg):
                # V rows scg*512..+512 (4 s-chunks per group)
                xvt = []
                for ch in range(KIN):
                    xt = xin.tile([128, 512], F32R, tag="xv", bufs=10,
                                  name=f"xv{scg}_{ch}")
                    nc.sync.dma_start(
                        xt[:],
                        xvT[ch * 128:(ch + 1) * 128,
                            scg * 512:(scg + 1) * 512].bitcast(F32R))
                    xvt.append(xt)
                for si in range(4):
                    sc = scg * 4 + si
                    psv = pso.tile([128, 2 * DC], F32, tag="pso",
                                   name=f"psv{sc}")
                    for ch in range(KIN):
                        nc.tensor.matmul(
                            psv[:], xvt[ch][:, si * 128:(si + 1) * 128],
                            wv_sb[:, ch, None, :].broadcast_to([128, 2, DC]),
                            start=(ch == 0), stop=(ch == KIN - 1))
                    nc.vector.tensor_add(
                        v_sb[:, sc, :, 0:DEPTH],
                        psv[:, 0:DC].rearrange("p (h d) -> p h d",
                                               h=HEADS_PER_CORE),
                        bv_sb[:].rearrange("p (h d) -> p h d",
                                           h=HEADS_PER_CORE))

            def emit_proj_half(xdram, w_sb, b_sb, dst, pi, sh):
                xch = []
                for ch in range(KIN):
                    xt = xin.tile([128, 1024], F32R, tag="x",
                                  name=f"x{pi}{sh}_{ch}")
                    nc.sync.dma_start(
                        xt[:],
                        xdram[ch * 128:(ch + 1) * 128,
                              sh * 1024:(sh + 1) * 1024].bitcast(F32R))
                    xch.append(xt)
                for m in range(2):
                    big = psbig.tile([128, 1024], F32, tag="big",
                                     name=f"pb{pi}{sh}{m}")
                    for ch in range(KIN):
                        for st in range(2):
                            nc.tensor.matmul(
                                big[:, st * QT:(st + 1) * QT],
                                w_sb[:, ch, m * 128:(m + 1) * 128],
                                xch[ch][:, st * QT:(st + 1) * QT],
                                start=(ch == 0), stop=(ch == KIN - 1))
                    nc.vector.tensor_scalar_add(
                        dst[m][:, sh * 1024:(sh + 1) * 1024], big[:],
                        b_sb[:, m:m + 1])

            def emit_attention(bi, t):
                qsl = slice(t * QT, (t + 1) * QT)
                pairs = []
                for pj in range(NPAIR):
                    v0, v1, mi = pair_plan[(t, pj)]
                    if v0 or v1:
                        pairs.append((pj, v0, v1, mi))
                n_valid = sum(int(v0) + int(v1) for _, v0, v1, _ in pairs)
                po = {}
                n_av = {0: 0, 1: 0}
                for hp in range(2):
                    po[hp] = pso.tile([DEPTH + 1, QT], F32, tag="pso",
                                      name=f"po{bi}{t}{hp}")
                exps = {}

                def emit_av(i):
                    pj, v0, v1, _ = pairs[i]
                    et = exps[i]
                    for hp in range(2):
                        h = 2 * bi + hp
                        for half, valid in ((0, v0), (1, v1)):
                            if not valid:
                                continue
                            kb = 2 * pj + half
                            nc.tensor.matmul(
                                po[hp][:],
                                v_sb[:, kb, h, :],
                                et[hp][:, half * QT:(half + 1) * QT],
                                start=(n_av[hp] == 0),
                                stop=(n_av[hp] == n_valid - 1))
                            n_av[hp] += 1

                for i, (pj, v0, v1, mi) in enumerate(pairs):
                    lg = {}
                    for hp in range(2):
                        lg[hp] = psbig.tile(
                            [128, 1024], F32, tag="big",
                            name=f"lg{bi}{t}{pj}{hp}")
                    for half, valid in ((0, v0), (1, v1)):
                        if not valid:
                            continue
                        kb = 2 * pj + half
                        for hp in range(2):
                            prow = slice(hp * 64, hp * 64 + 64)
                            nc.tensor.matmul(
                                lg[hp][:, half * QT:(half + 1) * QT],
                                kt_sb[bi][prow, kb * KB:(kb + 1) * KB],
                                qt_sb[bi][prow, qsl],
                                start=True, stop=True)
                    et = {}
                    for hp in range(2):
                        if mi is not None:
                            nc.vector.tensor_add(
                                lg[hp][:], lg[hp][:], mask_sb[mi][:])
                        et[hp] = expp.tile([128, 1024], F32R, tag="exp",
                                           name=f"et{bi}{t}{pj}{hp}")
                        if v0 and v1:
                            nc.scalar.activation(
                                et[hp][:], lg[hp][:],
                                mybir.ActivationFunctionType.Exp)
                        else:
                            half = 0 if v0 else 1
                            hs = slice(half * QT, (half + 1) * QT)
                            nc.scalar.activation(
                                et[hp][:, hs], lg[hp][:, hs],
                                mybir.ActivationFunctionType.Exp)
                    exps[i] = et
                    if i > 0:
                        emit_av(i - 1)
                if pairs:
                    emit_av(len(pairs) - 1)

                # extract O (unnormalized) and the denominator row
                for hp in range(2):
                    h = 2 * bi + hp
                    ht = h * NQT + t
                    ost = ostp.tile([DEPTH + 1, QT], F32, tag="ost",
                                    name=f"ost{bi}{t}{hp}")
                    nc.vector.tensor_copy(ost[:], po[hp][:])
                    nc.gpsimd.dma_start(
                        ot_sb[bi][hp * 64:hp * 64 + 64, qsl],
                        ost[0:DEPTH, :].bitcast(F32R))
                    src = ost[DEPTH:DEPTH + 1, :].rearrange(
                        "o (p j) -> o p j", j=4)
                    nc.gpsimd.dma_start(rs_sb[:, ht * 4:(ht + 1) * 4], src)

            def emit_norm(bi, tp):
                # normalize ot_sb[bi][:, tp*1024:(tp+1)*1024] (query tiles
                # 2*tp and 2*tp+1)
                HW = 2 * QT
                bcb = rowp.tile([128, HW], F32, tag="bcb",
                                name=f"bcb{bi}{tp}")
                for hp in range(2):
                    h = 2 * bi + hp
                    c0 = (h * NQT + 2 * tp) * 4
                    nc.vector.reciprocal(rr_sb[:, c0:c0 + 8],
                                         rs_sb[:, c0:c0 + 8])
                    rowh = rowp.tile([1, HW], F32, tag="rowh",
                                     name=f"rowh{bi}{tp}{hp}")
                    for ti in range(2):
                        c = c0 + ti * 4
                        nc.gpsimd.dma_start(
                            rowh[0:1, ti * QT:(ti + 1) * QT].rearrange(
                                "o (p j) -> o p j", j=4),
                            rr_sb[:, c:c + 4])
                    if hp == 0:
                        nc.gpsimd.partition_broadcast(bcb[0:64, :], rowh[:])
                    else:
                        tmp = rowp.tile([64, HW], F32, tag="tmp",
                                        name=f"tmp{bi}{tp}")
                        nc.gpsimd.partition_broadcast(tmp[:], rowh[:])
                        nc.gpsimd.dma_start(bcb[64:128, :], tmp[:])
                csl = slice(tp * HW, (tp + 1) * HW)
                nc.vector.tensor_mul(ot_sb[bi][:, csl], ot_sb[bi][:, csl],
                                     bcb[:])

            def emit_outproj(dt, sh):
                big = psbig.tile([128, 1024], F32, tag="big",
                                 name=f"pp{dt}{sh}")
                for bi in range(2):
                    for st in range(2):
                        col = sh * 1024 + st * QT
                        nc.tensor.matmul(
                            big[:, st * QT:(st + 1) * QT],
                            wo_sb[:, bi, dt * 128:(dt + 1) * 128],
                            ot_sb[bi][:, col:col + QT],
                            start=(bi == 0), stop=(bi == 1))
                ost = xin.tile([128, 1024], F32, tag="x",
                               name=f"os{dt}{sh}")
                if (dt + sh) % 2 == 0:
                    nc.vector.tensor_copy(ost[:], big[:])
                else:
                    nc.scalar.copy(ost[:], big[:])
                nc.gpsimd.dma_start(
                    outT[dt * 128:(dt + 1) * 128,
                         sh * 1024:(sh + 1) * 1024], ost[:])

            # ---- driver: overlap attention/outproj with projections -------
            emit_proj_half(xqT, wq_sb, bq_sb, qt_sb, 0, 0)
            emit_v_group(0)
            emit_v_group(1)
            emit_proj_half(xkT, wk_sb, bk_sb, kt_sb, 1, 0)
            for bi in range(2):
                for t in (0, 1):
                    emit_attention(bi, t)
            emit_norm(0, 0)
            emit_norm(1, 0)
            emit_v_group(2)
            emit_v_group(3)
            emit_proj_half(xqT, wq_sb, bq_sb, qt_sb, 0, 1)
            emit_proj_half(xkT, wk_sb, bk_sb, kt_sb, 1, 1)
            emit_attention(0, 2)
            for dt in range(4):
                emit_outproj(dt, 0)
            emit_attention(1, 2)
            for dt in range(4, 8):
                emit_outproj(dt, 0)
            emit_attention(0, 3)
            emit_norm(0, 1)
            emit_attention(1, 3)
            emit_norm(1, 1)
            for dt in range(8):
                emit_outproj(dt, 1)

    nc.compile()
    return nc


def _plan_from_mask(mask):
    """Classify (qtile, kblock-pair) blocks; return plan + unique pair tiles.

    pair_plan[(t, pj)] = (valid0, valid1, mask_idx|None); mask tiles are
    additive [128, 1024] (transposed mask halves scaled by NEG).
    """
    m = np.asarray(mask).reshape(S, S)  # [q, k]
    plan = {}
    tiles = []
    keys = {}
    for t in range(NQT):
        for pj in range(NPAIR):
            halves = []
            for half in range(2):
                kb = 2 * pj + half
                blk = m[t * QT:(t + 1) * QT, kb * KB:(kb + 1) * KB]  # [q,k]
                if not blk.any():
                    halves.append("plain")
                elif (blk == 1.0).all():
                    halves.append("skip")
                else:
                    halves.append(np.ascontiguousarray(
                        blk.T.astype(np.float32) * NEG))
            v0 = not (isinstance(halves[0], str) and halves[0] == "skip")
            v1 = not (isinstance(halves[1], str) and halves[1] == "skip")
            if not (v0 or v1):
                plan[(t, pj)] = (False, False, None)
                continue
            if all(isinstance(h, str) for h in halves):
                plan[(t, pj)] = (v0, v1, None)
                continue
            pair = np.zeros((KB, 2 * QT), np.float32)
            for half in range(2):
                hv = halves[half]
                if not isinstance(hv, str):
                    pair[:, half * QT:(half + 1) * QT] = hv
            key = pair.tobytes()
            if key not in keys:
                keys[key] = len(tiles)
                tiles.append(pair)
            plan[(t, pj)] = (v0, v1, keys[key])
    return plan, tiles


def kernel(query, key_in, value, mask, wq, bq, wk, bk, wv, bv, wo, bo):
    query = np.asarray(query, dtype=np.float32)
    key_in = np.asarray(key_in, dtype=np.float32)
    value = np.asarray(value, dtype=np.float32)
    wq = np.asarray(wq, dtype=np.float32)
    wk = np.asarray(wk, dtype=np.float32)
    wv = np.asarray(wv, dtype=np.float32)
    wo = np.asarray(wo, dtype=np.float32)
    bq = np.asarray(bq, dtype=np.float32)
    bk = np.asarray(bk, dtype=np.float32)
    bv = np.asarray(bv, dtype=np.float32)
    bo = np.asarray(bo, dtype=np.float32)

    plan, mask_tiles = _plan_from_mask(mask)
    sig = tuple(sorted(plan.items()))
    if sig not in _cache:
        _cache[sig] = _build(plan, len(mask_tiles))
    nc = _cache[sig]

    scale = 1.0 / np.sqrt(np.float32(DEPTH))
    masks_arr = (np.stack(mask_tiles) if mask_tiles
                 else np.zeros((1, KB, 2 * QT), np.float32))

    xT = {}
    for b in range(B):
        xT[("q", b)] = np.ascontiguousarray(query[b].T)
        xT[("k", b)] = np.ascontiguousarray(key_in[b].T)
        xT[("v", b)] = np.ascontiguousarray(value[b].T)

    in_maps = []
    for c in range(N_CORES):
        b = c // CORES_PER_BATCH
        g = c % CORES_PER_BATCH
        sl = slice(g * DC, (g + 1) * DC)
        in_maps.append({
            "xqT": xT[("q", b)],
            "xkT": xT[("k", b)],
            "xvT": xT[("v", b)],
            "wq": np.ascontiguousarray(wq[:, sl]) * scale,
            "wk": np.ascontiguousarray(wk[:, sl]),
            "wv": np.ascontiguousarray(wv[:, sl]),
            "wo": np.ascontiguousarray(wo[sl, :]),
            "bq": np.ascontiguousarray((bq[sl] * scale).reshape(2, 128).T),
            "bk": np.ascontiguousarray(bk[sl].reshape(2, 128).T),
            "bv": np.ascontiguousarray(
                np.broadcast_to(bv[sl], (128, DC))),
            "masks": masks_arr,
        })

    res = run_bass_kernel_spmd(nc, in_maps, list(range(N_CORES)))
    kernel.last_results = res

    out = np.zeros((B, S, D_MODEL), np.float32)
    for b in range(B):
        acc = np.zeros((D_MODEL, S), np.float32)
        for g in range(CORES_PER_BATCH):
            acc += res.results[b * CORES_PER_BATCH + g]["outT"]
        out[b] = acc.T + bo
    return out
